# revision 1
# baseline (speedup 1.0000x reference)
"""Locally-connected network (28x28 -> lc3x3 -> lc3x3 -> fc10) on 8 TRN2 cores.

The whole reference network is linear (two locally-connected layers + FC, no
activations), so on the host we fold it into a single affine map
    out[b, :] = x[b, :784] @ M + c          (M: [784, 10], c: [10])
computed in float64. The device kernel is then a pure data-parallel,
memory-bound matmul: each of the 8 cores streams its 1024-sample shard of x
(transposed to pixel-major [784, 1024] on host) and does 7 accumulating
K-tile matmuls into PSUM, adds the bias via ScalarE, and writes [10, 1024].
"""

import numpy as np

import concourse.bass as bass
import concourse.tile as tile
from concourse import bacc, mybir
from concourse.bass_utils import run_bass_kernel_spmd

N_CORES = 8
B = 8192
B_SHARD = B // N_CORES          # 1024
PIX = 784                       # 28*28
KP = 112                        # K-tile partition count; 784 = 7 * 112
NKT = PIX // KP                 # 7
NCHUNK = 2                      # batch chunks of 512 (PSUM bank = 512 f32)
CH = B_SHARD // NCHUNK          # 512
NOUT = 10
MW_COLS = 128                   # padded free dim (512B rows); cols 0..69 = M, col 70 = bias


def _lc_dense(w, H, W_, oh, ow):
    """Dense [H*W_, oh*ow] matrix of one 3x3 locally-connected layer."""
    w = np.asarray(w, np.float64).reshape(oh, ow, 9)
    M = np.zeros((H * W_, oh * ow), np.float64)
    ox, oy = np.meshgrid(np.arange(oh), np.arange(ow), indexing="ij")
    col = (ox * ow + oy).ravel()
    for i in range(3):
        for j in range(3):
            row = ((ox + i) * W_ + (oy + j)).ravel()
            M[row, col] += w[:, :, i * 3 + j].ravel()
    return M


def _fold(w1, b1, w2, b2, fc_w, fc_b):
    W1 = _lc_dense(w1, 28, 28, 26, 26)          # [784, 676]
    W2 = _lc_dense(w2, 26, 26, 24, 24)          # [676, 576]
    fcw = np.asarray(fc_w, np.float64)          # [10, 576]
    M = W1 @ W2 @ fcw.T                         # [784, 10]
    c = (
        np.asarray(b1, np.float64).reshape(-1) @ W2
        + np.asarray(b2, np.float64).reshape(-1)
    ) @ fcw.T + np.asarray(fc_b, np.float64)    # [10]
    return M.astype(np.float32), c.astype(np.float32)


def _build_bass():
    nc = bacc.Bacc("TRN2", target_bir_lowering=False, debug=False)
    xt = nc.declare_dram_parameter("xt", [KP, NKT, B_SHARD], mybir.dt.float32, isOutput=False)
    mw = nc.declare_dram_parameter("mw", [KP, MW_COLS], mybir.dt.float32, isOutput=False)
    out = nc.declare_dram_parameter("out", [NOUT, B_SHARD], mybir.dt.float32, isOutput=True)

    with tile.TileContext(nc) as tc:
        with (
            tc.tile_pool(name="wp", bufs=1) as wp,
            tc.tile_pool(name="xp", bufs=NKT) as xp,
            tc.tile_pool(name="pp", bufs=NCHUNK, space="PSUM") as pp,
            tc.tile_pool(name="op", bufs=NCHUNK) as op,
        ):
            m_sb = wp.tile([KP, MW_COLS], mybir.dt.float32)
            nc.sync.dma_start(m_sb[:], mw[:])

            # TRN2 LDWEIGHTS lowering allows a single sync wait; a matmul
            # whose operands arrive via two DMA lanes fails codegen ("too
            # many sync wait commands"). Absorb the m_sb wait on PE with a
            # throwaway matmul that only reads m_sb, so every real matmul
            # waits on at most its own x-tile lane.
            warm = pp.tile([NOUT, 1], mybir.dt.float32)
            nc.tensor.matmul(
                warm[:], m_sb[:, 0:NOUT], m_sb[:, 0:1], start=True, stop=True
            )
            # Same single-wait constraint on ScalarE: the bias-add below reads
            # both PSUM (PE sem) and m_sb (DMA lane); touch m_sb here so the
            # real activation only waits on the PE sem.
            scratch = op.tile([1, 1], mybir.dt.float32)
            nc.scalar.copy(scratch[:], m_sb[0:1, 0:1])

            # Keep total DMA count <= 8 so no DMAHW semaphore lane is reused
            # (lane reuse adds a second sync wait to a DMA, which TRN2
            # codegen rejects). 4 x-loads + m_sb + 1 output = 6 lanes.
            # x is packed [KP, NKT, B] on host so each partition reads
            # contiguous 8KB per 2-k-tile group; loads alternate between the
            # two HWDGE rings (sync / scalar) to double descriptor feed rate.
            groups = [(0, 2), (2, 2), (4, 2), (6, 1)]  # (first kt, n k-tiles)
            rings = [nc.sync, nc.sync, nc.sync, nc.sync]
            xts = [None] * NKT
            for (k0, nk), ring in zip(groups, rings):
                t = xp.tile([KP, nk, B_SHARD], mybir.dt.float32)
                ring.dma_start(t[:], xt[:, k0 : k0 + nk, :])
                for j in range(nk):
                    xts[k0 + j] = (t, j)

            o = op.tile([NOUT, B_SHARD], mybir.dt.float32)
            for ch in range(NCHUNK):
                ps = pp.tile([NOUT, CH], mybir.dt.float32)
                for kt in range(NKT):
                    t, j = xts[kt]
                    nc.tensor.matmul(
                        ps[:],
                        m_sb[:, kt * NOUT : (kt + 1) * NOUT],
                        t[:, j, ch * CH : (ch + 1) * CH],
                        start=(kt == 0),
                        stop=(kt == NKT - 1),
                    )
                nc.scalar.activation(
                    o[:, ch * CH : (ch + 1) * CH],
                    ps[:],
                    mybir.ActivationFunctionType.Identity,
                    bias=m_sb[0:NOUT, 70:71],
                )
            nc.sync.dma_start(out[:], o[:])
    nc.finalize()
    return nc


def _run(inputs, trace=False, trace_cores=None):
    x = np.asarray(inputs["x"], np.float32)
    M, c = _fold(
        inputs["w1"], inputs["b1"], inputs["w2"], inputs["b2"],
        inputs["fc_w"], inputs["fc_b"],
    )
    mp = np.zeros((KP, MW_COLS), np.float32)
    for kt in range(NKT):
        mp[:, kt * NOUT : (kt + 1) * NOUT] = M[kt * KP : (kt + 1) * KP]
    mp[0:NOUT, 70] = c

    # Pack per-core shard to [KP, NKT, B_SHARD]: xt[p, kt, b] = x[b, kt*KP+p],
    # so every partition's k-tile group is one contiguous DRAM read.
    xr = x.reshape(B, PIX)
    in_maps = [
        {
            "xt": np.ascontiguousarray(
                xr[i * B_SHARD : (i + 1) * B_SHARD]
                .reshape(B_SHARD, NKT, KP)
                .transpose(2, 1, 0)
            ),
            "mw": mp,
        }
        for i in range(N_CORES)
    ]

    nc = _build_bass()
    res = run_bass_kernel_spmd(
        nc,
        in_maps,
        list(range(N_CORES)),
        trace=trace,
        trace_cores=trace_cores,
    )
    out = np.concatenate(
        [np.asarray(res.results[i]["out"]).T for i in range(N_CORES)], axis=0
    ).astype(np.float32)
    return out, res


def kernel(**inputs) -> np.ndarray:
    out, _ = _run(inputs, trace=False)
    return out



# revision 13
# speedup vs baseline: 3.8446x; 3.8446x over previous
"""Locally-connected network (28x28 -> lc3x3 -> lc3x3 -> fc10) on 8 TRN2 cores.

The whole reference network is linear (two locally-connected layers + FC, no
activations), so on the host we fold it into a single affine map
    out[b, :] = x[b, :784] @ M + c          (M: [784, 10], c: [10])
computed in float64. The device kernel is a pure data-parallel, memory-bound
matmul over each core's 1024-sample shard.

Device-side layout (per core), all fp16 to halve HBM traffic (quantization
error ~1e-3 relative, far inside the 2e-2 gate):
  xt: [112, 128 + 7*1024] — a 128-col weight block (7 k-tile blocks of M's 10
      output columns; bias c in row 0, cols 71..80), then batch slices laid
      out k-tile-major so one DMA per slice brings all 7 k-tiles
      contiguously per partition.
The weight block rides in slice 0's DMA so every instruction waits on at
most one DMA semaphore lane (TRN2 codegen rejects multi-wait consumers).
Per slice: 7 accumulating [112,10]x[112,SL] matmuls plus a [1,10]x[1,SL]
ones-row matmul that folds the bias into PSUM (so no serialized ScalarE
bias-add chain), then a PSUM->SBUF fp16 downcast copy. Copies for all but
the last slice run on DVE, the last on ScalarE, so the drain after the last
(small) slice overlaps the streaming-out of earlier slices.
"""

import numpy as np

import concourse.bass as bass
import concourse.tile as tile
from concourse import bacc, mybir
from concourse.bass_utils import run_bass_kernel_spmd

N_CORES = 8
B = 8192
B_SHARD = B // N_CORES          # 1024
PIX = 784                       # 28*28
KP = 112                        # K-tile partition count; 784 = 7 * 112
NKT = PIX // KP                 # 7
# Batch-slice sizes: the last slice is tiny so the post-stream drain
# (matmuls + copy + store for it) is short.
SLICES = (352, 448, 192, 32)
NSLICE = len(SLICES)
SOFF = tuple(sum(SLICES[:i]) for i in range(NSLICE + 1))  # batch offsets
MWC = 128                       # weight block cols
NCOLS = MWC + NKT * B_SHARD     # 7296
NOUT = 10
BIAS_COL = 71                   # c at [0, 71:81]


def _lc_dense(w, H, W_, oh, ow):
    """Dense [H*W_, oh*ow] matrix of one 3x3 locally-connected layer."""
    w = np.asarray(w, np.float64).reshape(oh, ow, 9)
    M = np.zeros((H * W_, oh * ow), np.float64)
    ox, oy = np.meshgrid(np.arange(oh), np.arange(ow), indexing="ij")
    col = (ox * ow + oy).ravel()
    for i in range(3):
        for j in range(3):
            row = ((ox + i) * W_ + (oy + j)).ravel()
            M[row, col] += w[:, :, i * 3 + j].ravel()
    return M


def _fold(w1, b1, w2, b2, fc_w, fc_b):
    W1 = _lc_dense(w1, 28, 28, 26, 26)          # [784, 676]
    W2 = _lc_dense(w2, 26, 26, 24, 24)          # [676, 576]
    fcw = np.asarray(fc_w, np.float64)          # [10, 576]
    M = W1 @ W2 @ fcw.T                         # [784, 10]
    c = (
        np.asarray(b1, np.float64).reshape(-1) @ W2
        + np.asarray(b2, np.float64).reshape(-1)
    ) @ fcw.T + np.asarray(fc_b, np.float64)    # [10]
    return M.astype(np.float32), c.astype(np.float32)


def _build_bass():
    nc = bacc.Bacc("TRN2", target_bir_lowering=False, debug=False)
    f16 = mybir.dt.float16
    f32 = mybir.dt.float32
    xt = nc.declare_dram_parameter("xt", [KP, NCOLS], f16, isOutput=False)
    out = nc.declare_dram_parameter("out", [NOUT, B_SHARD], f16, isOutput=True)

    with tile.TileContext(nc) as tc:
        with (
            tc.tile_pool(name="xp", bufs=NSLICE) as xp,
            tc.tile_pool(name="pp", bufs=NSLICE, space="PSUM") as pp,
            tc.tile_pool(name="wp", bufs=2, space="PSUM") as wp,
            tc.tile_pool(name="op", bufs=2) as op,
        ):
            # Ones row for the bias-fold matmul; DVE memset, no input deps.
            ones = op.tile([1, max(SLICES)], f16)
            nc.vector.memset(ones[:], 1.0)

            # Slice 0's DMA also carries the weight block.
            t0 = xp.tile([KP, MWC + NKT * SLICES[0]], f16)
            nc.sync.dma_start(t0[:], xt[:, 0 : MWC + NKT * SLICES[0]])

            # Absorb the t0-DMA and ones-memset waits once on PE so real
            # matmuls wait on at most one semaphore lane each.
            warm = wp.tile([NOUT, 1], f32)
            nc.tensor.matmul(
                warm[:], t0[:, 0:NOUT], t0[:, 0:1], start=True, stop=True
            )
            warm2 = wp.tile([1, 1], f32)
            nc.tensor.matmul(
                warm2[:], ones[0:1, 0:1], ones[0:1, 0:1], start=True, stop=True
            )

            xs = [t0]
            for s in range(1, NSLICE):
                t = xp.tile([KP, NKT * SLICES[s]], f16)
                nc.sync.dma_start(
                    t[:],
                    xt[:, MWC + NKT * SOFF[s] : MWC + NKT * SOFF[s + 1]],
                )
                xs.append(t)

            o = op.tile([NOUT, B_SHARD], f16)
            for s in range(NSLICE):
                base = MWC if s == 0 else 0
                sl = SLICES[s]
                ps = pp.tile([NOUT, sl], f32)
                for kt in range(NKT):
                    nc.tensor.matmul(
                        ps[:],
                        t0[:, kt * NOUT : (kt + 1) * NOUT],
                        xs[s][:, base + kt * sl : base + (kt + 1) * sl],
                        start=(kt == 0),
                        stop=False,
                    )
                nc.tensor.matmul(
                    ps[:],
                    t0[0:1, BIAS_COL : BIAS_COL + NOUT],
                    ones[0:1, 0:sl],
                    start=False,
                    stop=True,
                )
                dst = o[:, SOFF[s] : SOFF[s + 1]]
                if s < NSLICE - 1:
                    nc.vector.tensor_scalar_add(dst, ps[:], 0.0)
                else:
                    nc.scalar.copy(dst, ps[:])
                if s == 1:
                    nc.sync.dma_start(out[:, 0 : SOFF[2]], o[:, 0 : SOFF[2]])
                elif s == 2:
                    nc.sync.dma_start(out[:, SOFF[2] : SOFF[3]], o[:, SOFF[2] : SOFF[3]])
            # Tail store rides the scalar ring right behind the final copy:
            # engine-ordered, no cross-engine wait.
            nc.scalar.dma_start(
                out[:, SOFF[NSLICE - 1] :], o[:, SOFF[NSLICE - 1] :]
            )
    nc.finalize()
    return nc


def _prepare(inputs):
    x = np.asarray(inputs["x"], np.float32)
    M, c = _fold(
        inputs["w1"], inputs["b1"], inputs["w2"], inputs["b2"],
        inputs["fc_w"], inputs["fc_b"],
    )
    mp = np.zeros((KP, MWC), np.float16)
    for kt in range(NKT):
        mp[:, kt * NOUT : (kt + 1) * NOUT] = M[kt * KP : (kt + 1) * KP]
    mp[0, BIAS_COL : BIAS_COL + NOUT] = c

    xr = x.reshape(B, PIX).astype(np.float16)
    in_maps = []
    for i in range(N_CORES):
        shard = xr[i * B_SHARD : (i + 1) * B_SHARD]        # [1024, 784]
        arr = np.empty((KP, NCOLS), np.float16)
        arr[:, 0:MWC] = mp
        # Per slice s: [p, MWC + NKT*SOFF[s] + kt*SLICES[s] + b]
        #            = shard[SOFF[s] + b, kt*KP + p]
        for s in range(NSLICE):
            arr[:, MWC + NKT * SOFF[s] : MWC + NKT * SOFF[s + 1]] = (
                shard[SOFF[s] : SOFF[s + 1]]
                .reshape(SLICES[s], NKT, KP)
                .transpose(2, 1, 0)
                .reshape(KP, NKT * SLICES[s])
            )
        in_maps.append({"xt": arr})
    return in_maps


def _build_for_sim(inputs):
    return _build_bass(), _prepare(inputs)[0]


def _run(inputs, trace=False, trace_cores=None):
    in_maps = _prepare(inputs)
    nc = _build_bass()
    res = run_bass_kernel_spmd(
        nc,
        in_maps,
        list(range(N_CORES)),
        trace=trace,
        trace_cores=trace_cores,
    )
    out = np.concatenate(
        [np.asarray(res.results[i]["out"]).T for i in range(N_CORES)], axis=0
    ).astype(np.float32)
    return out, res


def kernel(**inputs) -> np.ndarray:
    out, _ = _run(inputs, trace=False)
    return out
